# revision 19
# baseline (speedup 1.0000x reference)
"""DiffPool GNN forward on 8 Trainium2 NeuronCores.

Data-parallel over the batch dim (B=16 -> 2 batches per core). Each core
receives its two batches' dense node features (bf16, transposed) and dense
transposed adjacency (fp8e4, 0/1 exact), runs the DiffPool pipeline locally,
and emits its two [2] outputs.

Key structure (v3):
  - the dominant GEMMs (t = adj@s, a1t = t^T s, x1t = h^T s) run in fp8e4m3
    with MatmulPerfMode.DoubleRow: both operands pack two 128-deep k-tiles
    along a middle free dim, the PE streams at 0.5 cycles per output column
    (4x fewer streaming cycles than bf16 per contraction).
  - softmax scores s (~1/205) and embeddings h (~0.03) sit in e4m3's
    subnormal range, so they are scaled by 64 and 16 (powers of 2) before
    quantization; the epilogue copies divide the scale back out exactly.
    t is fp8 too; a1/x1 contract 2048 rows so per-element fp8 noise
    averages out (numpy sim: 5.4e-3 end-to-end rel err).
  - one proj matmul per node-chunk produces scores AND embeddings in a
    single 272-wide psum (exp reads cols 0:205, relu-pack reads 208:272).
  - the two level-2 GNNs (s2-head, 21 wide, and x1e-head, 64 wide) share
    every lhsT, so they run as ONE 85-wide chain using block-diagonal
    packed weights — half the matmuls and half the copies of the tail.
  - input DMA is phase-serialized (weights+x first, then batch-0 slabs,
    then batch-1 slabs) via tiny barrier DMAs: the DMA ring round-robins
    across live transfers, so without the barriers 6.6MB of adjacency
    starves the small latency-critical weight transfers.
  - elementwise work is spread across ACT (exp), DVE (relu-pack, recip,
    norm b0, t-copies b0, descales) and Pool (pair softmax sums, norm b1),
    with t-copies b1 on ACT after its exps are done.
"""

import numpy as np
import ml_dtypes

import concourse.bass as bass
import concourse.mybir as mybir
from concourse import tile
from concourse.bass_utils import run_bass_kernel_spmd

# ---------------------------------------------------------------------------
# Problem constants (hardcoded per spec)
# ---------------------------------------------------------------------------
B = 16
NCORES = 8
BPC = B // NCORES          # batches per core
MN = 2048                  # MAX_NODES
IN_DIM = 128
HID = 64
OUT = 2
K1 = 205
K2 = 21
N1P = 1100                 # g1 nodes per batch
N2P = 900                  # g2 nodes per batch
W1COLS = 1152              # trimmed slab width for dst-chunks 0..8
PROJ_N = 272               # 205 scores | 3 pad | 64 emb  (relu offset 8B-aligned)
HOFF = 208                 # emb column offset inside proj psum
L2C = K2 + HID             # 85: combined s2|x1e level-2 width

SSCALE = 64.0              # fp8 pre-scale for softmax scores
HSCALE = 16.0              # fp8 pre-scale for relu embeddings

F32 = mybir.dt.float32
BF16 = mybir.dt.bfloat16
FP8 = mybir.dt.float8e4
DR = mybir.MatmulPerfMode.DoubleRow

_M2 = ((0, 128), (128, K1 - 128))      # row tiling of a 205-row matrix


# ---------------------------------------------------------------------------
# Walrus workaround: this toolchain's walrus encodes at most ONE sync wait
# per instruction (single EVENTS slot) and errors out instead of splitting.
# Split any multi-wait instruction by hoisting extra waits onto fresh
# single-wait NOPs right before it on the same engine.
# ---------------------------------------------------------------------------
_mw_ctr = [0]


def _legalize_multiwait(nc):
    for func in nc.m.functions:
        for bb in func.blocks:
            insts = bb.instructions
            new = []
            changed = False
            for ins in insts:
                si = getattr(ins, "sync_info", None)
                waits = list(si.on_wait) if (si and si.on_wait) else []
                if len(waits) > 1:
                    changed = True
                    for w in waits[:-1]:
                        _mw_ctr[0] += 1
                        nop = mybir.InstNoOp(
                            name=f"mwfix-{_mw_ctr[0]}",
                            engine=ins.engine,
                            ins=[],
                            outs=[],
                            sync_info=mybir.SyncInfo(on_wait=[w], on_update=[]),
                            bass_nofuse=True,
                        )
                        nc.register_instruction(nop, overwrite=True)
                        new.append(nop)
                    si.on_wait = [waits[-1]]
                new.append(ins)
            if changed:
                bb.instructions[:] = new


# ---------------------------------------------------------------------------
# Device program
# ---------------------------------------------------------------------------
def build_nc():
    nc = bass.Bass()

    # packed features: per batch, [xt1 (node cols 0..1151, g1+mixed) |
    # xt2 (cols 1024..2047 at offset -1024, mixed+g2)]; batch 0 also
    # carries the projection weights so phase-1 DMA is a single transfer.
    xp0 = nc.dram_tensor("xp0", [IN_DIM, 17 * 128 + 2 * PROJ_N], BF16,
                         kind="ExternalInput")
    xp1 = nc.dram_tensor("xp1", [IN_DIM, 17 * 128], BF16,
                         kind="ExternalInput")
    adj1 = nc.dram_tensor("adj1", [BPC, 8, 128, W1COLS], FP8, kind="ExternalInput")
    adj2 = nc.dram_tensor("adj2", [BPC, 8, 128, MN], FP8, kind="ExternalInput")
    # combined level-2 weights: [W1c | U1c | W2c | U2c], each 85 cols;
    # W1c/U1c have 64 live rows, W2c/U2c are 85x85 block-diagonal
    wl2cat = nc.dram_tensor("wl2cat", [L2C, 4 * L2C], BF16,
                            kind="ExternalInput")
    # f32 copies of the final-GNN weights (the late stages run f32: their
    # matmuls are tiny but their values are huge, so bf16 rounding there
    # dominates the error budget)
    wl2f32 = nc.dram_tensor("wl2f32", [HID, 2 * HID + 2 * OUT], F32,
                            kind="ExternalInput")
    scr_bf = nc.dram_tensor("scr_bf", [16], BF16, kind="Internal")
    scr_bf2 = nc.dram_tensor("scr_bf2", [16], BF16, kind="Internal")
    scr_f8 = nc.dram_tensor("scr_f8", [16], FP8, kind="Internal")
    out = nc.dram_tensor("out", [OUT, BPC], F32, kind="ExternalOutput")

    with tile.TileContext(nc) as tc:
        with (
            tc.tile_pool(name="const", bufs=1) as cpool,
            tc.tile_pool(name="xt", bufs=2) as xtpool,
            tc.tile_pool(name="slab", bufs=2) as slabpool,
            tc.tile_pool(name="work", bufs=2) as work,
            tc.tile_pool(name="acc", bufs=2, space="PSUM") as accp,
            tc.tile_pool(name="ps", bufs=4, space="PSUM") as ps,
        ):
            shpool = l2pool = smx = work
            # ---- DMA phase 1: packed features+weights (small, urgent).
            # The DMA ring round-robins across all live transfers, so the
            # slab bulk is held back behind barrier DMAs below. ----
            xp0_sb = xtpool.tile([IN_DIM, 17 * 128 + 2 * PROJ_N], BF16,
                                 tag="xp0")
            nc.sync.dma_start(out=xp0_sb[:], in_=xp0[:])
            xp1_sb = xtpool.tile([IN_DIM, 17 * 128], BF16, tag="xp1")
            nc.gpsimd.dma_start(out=xp1_sb[:], in_=xp1[:])
            wp1_sb = xp0_sb[:, 17 * 128:17 * 128 + PROJ_N]
            wp2_sb = xp0_sb[:, 17 * 128 + PROJ_N:]
            xt_sb = [(xp0_sb[:, :9 * 128], xp0_sb[:, 9 * 128:17 * 128]),
                     (xp1_sb[:, :9 * 128], xp1_sb[:, 9 * 128:17 * 128])]

            wcat_sb = cpool.tile([L2C, 4 * L2C], BF16, tag="wl2")
            nc.scalar.dma_start(out=wcat_sb[:], in_=wl2cat[:])
            wf_sb = cpool.tile([HID, 2 * HID + 2 * OUT], F32, tag="wf")
            nc.scalar.dma_start(out=wf_sb[:], in_=wl2f32[:])

            # ---- DMA phase 2+3: adjacency slabs, gated per batch; sl2
            # ships BEFORE sl1 (t-passes contract sl2 pairs first). The
            # barrier dma_start READS the previous phase's last tile, so its
            # trigger (and everything after it on the sync queue) waits for
            # that transfer to finish before the slabs enter the ring. ----
            slabs = []
            sl_t = [(slabpool.tile([128, 8, W1COLS], FP8, tag="s1",
                                   name=f"slab1_{b}"),
                     slabpool.tile([128, 8, MN], FP8, tag="s2",
                                   name=f"slab2_{b}"))
                    for b in range(BPC)]
            # one barrier per trigger engine: slabs enter the ring only
            # after the packed phase-1 transfers have fully landed (the
            # ring round-robins across live transfers, so the bulk would
            # starve the small critical ones). Both batches' slabs then
            # stream CONCURRENTLY, split in halves for ring parallelism —
            # the tail's gate is slab(1), not slab(0).
            nc.sync.dma_start(out=scr_bf[:], in_=xp0_sb[0, 0:16])
            nc.gpsimd.dma_start(out=scr_bf2[:], in_=xp0_sb[0:1, 16:32])
            for b in range(BPC):
                sl1, sl2 = sl_t[b]
                eng = nc.sync if b == 0 else nc.gpsimd
                for v0 in (0, 4):
                    eng.dma_start(
                        out=sl1[:, v0:v0 + 4, :],
                        in_=adj1[b, v0:v0 + 4].rearrange("v p u -> p v u"))
                for v0 in (0, 4):
                    eng.dma_start(
                        out=sl2[:, v0:v0 + 4, :],
                        in_=adj2[b, v0:v0 + 4].rearrange("v p u -> p v u"))
                slabs.append((sl1, sl2))

            W1c = wcat_sb[:HID, 0:L2C]
            U1c = wcat_sb[:HID, L2C:2 * L2C]
            W2c = wcat_sb[:, 2 * L2C:3 * L2C]
            U2c = wcat_sb[:, 3 * L2C:4 * L2C]
            wsb = {}
            wsb["Wc1f"] = wf_sb[:, :HID]
            wsb["Uc1f"] = wf_sb[:, HID:2 * HID]
            wsb["Wc2f"] = wf_sb[:, 2 * HID:2 * HID + OUT]
            wsb["Uc2f"] = wf_sb[:, 2 * HID + OUT:]
            warm = cpool.tile([128, 128], BF16, tag="warm")
            nc.gpsimd.memset(warm[:], 0.0)
            ones_col = cpool.tile([K2, 1], F32, tag="ones_col")
            nc.gpsimd.memset(ones_col[:], 1.0)
            out_sb = cpool.tile([OUT, BPC], F32, tag="out_sb")

            # ---- warmup: trigger the ACT exp table load now, and ramp the
            # PE p-state while the input DMAs land ----
            actwarm = cpool.tile([128, 1], F32, tag="actwarm")
            nc.scalar.activation(out=actwarm[:], in_=warm[:, 0:1],
                                 func=mybir.ActivationFunctionType.Exp,
                                 scale=1.0)
            for i in range(12):
                pw = ps.tile([128, 128], F32, tag="mm", name=f"warmps{i}")
                nc.tensor.matmul(pw[:], lhsT=warm[:], rhs=warm[:],
                                 start=True, stop=True)

            # ---- per-batch state (fp8 pair tiles: [:, i, :] = chunk 2p+i) ----
            S8 = [[None] * 8 for _ in range(BPC)]   # scaled softmax scores
            H8 = [[None] * 8 for _ in range(BPC)]   # scaled relu embeddings
            T8 = [[None] * 8 for _ in range(BPC)]   # scaled t = adj@s
            A1T = [[None, None] for _ in range(BPC)]
            X1T = [None] * BPC

            def _proj_mm(b, c, p):
                x1t_sb, x2t_sb = xt_sb[b]
                if c <= 7:
                    nc.tensor.matmul(p[:], lhsT=x1t_sb[:, c * 128:(c + 1) * 128],
                                     rhs=wp1_sb[:], start=True, stop=True)
                elif c == 8:
                    nc.tensor.matmul(p[:], lhsT=x1t_sb[:, 1024:1152],
                                     rhs=wp1_sb[:], start=True, stop=False)
                    nc.tensor.matmul(p[:], lhsT=x2t_sb[:, 0:128],
                                     rhs=wp2_sb[:], start=False, stop=True)
                else:
                    nc.tensor.matmul(p[:],
                                     lhsT=x2t_sb[:, (c - 8) * 128:(c - 7) * 128],
                                     rhs=wp2_sb[:], start=True, stop=True)

            def proj(b, norm_eng):
                """One 272-wide matmul per chunk -> scores (exp/softmax on
                cols 0:205) and embeddings (relu-pack on 208:272).
                Softmax: exp on ACT into bf16 pair tiles, pair-sum on Pool,
                pair-reciprocal on DVE, normalize-pack (x64, fp8) per chunk
                on norm_eng; relu-pack (x16, fp8) on DVE."""
                for pi in range(8):
                    S8[b][pi] = shpool.tile([128, 2, K1], FP8, tag="s8",
                                            bufs=16, name=f"s8_{b}_{pi}")
                    H8[b][pi] = shpool.tile([128, 2, HID], FP8, tag="h8",
                                            bufs=16, name=f"h8_{b}_{pi}")
                    ssum = smx.tile([128, 2], F32, tag="ssum", bufs=4,
                                    name=f"ss{b}_{pi}")
                    rinv = smx.tile([128, 2], F32, tag="rinv", bufs=4,
                                    name=f"ri{b}_{pi}")
                    sexp = shpool.tile([128, 2, K1], BF16, tag="sexp",
                                       bufs=4, name=f"se{b}_{pi}")
                    pp = []
                    for half in range(2):
                        c = 2 * pi + half
                        p = ps.tile([128, PROJ_N], F32, tag="mm",
                                    name=f"pj{b}_{c}")
                        _proj_mm(b, c, p)
                        pp.append(p)
                        nc.scalar.activation(
                            out=sexp[:, half, :], in_=p[:, :K1],
                            func=mybir.ActivationFunctionType.Exp,
                            scale=1.0)
                    nc.vector.tensor_reduce(out=ssum[:], in_=sexp[:],
                                            op=mybir.AluOpType.add,
                                            axis=mybir.AxisListType.X)
                    nc.vector.reciprocal(out=rinv[:], in_=ssum[:])
                    for half in range(2):
                        norm_eng.tensor_scalar(
                            out=S8[b][pi][:, half, :], in0=sexp[:, half, :],
                            scalar1=rinv[:, half:half + 1], scalar2=SSCALE,
                            op0=mybir.AluOpType.mult,
                            op1=mybir.AluOpType.mult)
                        nc.vector.tensor_scalar(
                            out=H8[b][pi][:, half, :],
                            in0=pp[half][:, HOFF:HOFF + HID],
                            scalar1=HSCALE, scalar2=0.0,
                            op0=mybir.AluOpType.mult,
                            op1=mybir.AluOpType.max)

            def _slab_pair(b, pi, usl):
                sl1, sl2 = slabs[b]
                if pi <= 3:
                    return sl1[:, 2 * pi:2 * pi + 2, usl]
                return sl2[:, 2 * pi - 8:2 * pi - 6, usl]

            def t_pass(b, u_lo):
                """u-chunks u_lo..u_lo+3 into two 2-bank PSUM pair tiles,
                DoubleRow fp8: each matmul contracts a PAIR of 128-src
                chunks. u-chunks <=8 contract pairs 0..7; >8 only pairs
                4..7 (g2 rows never see g1 columns)."""
                accs = [accp.tile([128, 2, 512], F32, tag="acc",
                                  name=f"acc{b}_{u_lo + 2 * i}")
                        for i in range(2)]
                for pi in range(8):
                    for i in range(4):
                        u = u_lo + i
                        p0 = 0 if u <= 8 else 4
                        if pi < p0:
                            continue
                        usl = slice(u * 128, (u + 1) * 128)
                        nc.tensor.matmul(accs[i // 2][:, i % 2, :K1],
                                         lhsT=_slab_pair(b, pi, usl),
                                         rhs=S8[b][pi][:],
                                         start=(pi == p0), stop=(pi == 7),
                                         perf_mode=DR)
                return accs

            def t_copies(b, accs, u_lo, eng):
                """One pass's PSUM pair accumulators -> fp8 t pair tiles
                (one 2-bank copy each, on the given engine)."""
                for i in range(2):
                    pi = u_lo // 2 + i
                    # padded to 208 cols: dual-fp8 LDWEIGHTS needs an even
                    # (aligned) inter-plane stride; 205 is rejected
                    T8[b][pi] = shpool.tile([128, 2, 208], FP8, tag="t8",
                                            bufs=16, name=f"t8_{b}_{pi}")
                    if eng is nc.scalar:
                        nc.scalar.activation(
                            out=T8[b][pi][:, :, :K1], in_=accs[i][:, :, :K1],
                            func=mybir.ActivationFunctionType.Copy)
                    else:
                        eng.tensor_copy(out=T8[b][pi][:, :, :K1],
                                        in_=accs[i][:, :, :K1])

            def epilogue(b):
                """a1t = t^T s  [205,205];  x1t = h^T s  [64,205] (bf16),
                DoubleRow over 8 n-chunk pairs, descaled on copy-out."""
                for mi, (m0, msz) in enumerate(_M2):
                    pa = ps.tile([128, K1], F32, tag="mm", name=f"pa1t{b}_{mi}")
                    for pi in range(8):
                        nc.tensor.matmul(pa[:msz, :],
                                         lhsT=T8[b][pi][:, :, m0:m0 + msz],
                                         rhs=S8[b][pi][:],
                                         start=(pi == 0), stop=(pi == 7),
                                         perf_mode=DR)
                    asb = l2pool.tile([128, K1], BF16, tag=f"a1t{mi}",
                                      name=f"a1t{b}_{mi}")
                    nc.vector.tensor_scalar_mul(out=asb[:msz, :],
                                                in0=pa[:msz, :],
                                                scalar1=1.0 / (SSCALE * SSCALE))
                    A1T[b][mi] = asb
                px = ps.tile([HID, K1], F32, tag="mm", name=f"px1t{b}")
                for pi in range(8):
                    nc.tensor.matmul(px[:], lhsT=H8[b][pi][:],
                                     rhs=S8[b][pi][:],
                                     start=(pi == 0), stop=(pi == 7),
                                     perf_mode=DR)
                xsb = l2pool.tile([HID, K1], BF16, tag="x1t", name=f"x1t{b}")
                nc.vector.tensor_scalar_mul(out=xsb[:], in0=px[:],
                                            scalar1=1.0 / (HSCALE * SSCALE))
                X1T[b] = xsb

            # =============== level-2: combined 85-wide s2|x1e chain =========
            def mm_chain(b, srcs, osh, tag, relu=False, dtype=BF16):
                p = ps.tile(list(osh), F32, tag="mm", name=f"p{tag}{b}")
                n = len(srcs)
                for i, (lt, rw) in enumerate(srcs):
                    nc.tensor.matmul(p[:], lhsT=lt, rhs=rw,
                                     start=(i == 0), stop=(i == n - 1))
                o = l2pool.tile(list(osh), dtype, tag=tag, name=f"{tag}{b}")
                if relu:
                    nc.scalar.activation(out=o[:], in_=p[:],
                                         func=mybir.ActivationFunctionType.Relu)
                else:
                    nc.vector.tensor_copy(out=o[:], in_=p[:])
                return o

            ctx = [dict() for _ in range(BPC)]

            def l2_z1(b):
                ctx[b]["z1"] = [
                    mm_chain(b, [(X1T[b][:, m0:m0 + msz], W1c)],
                             (msz, L2C), f"z1c_{mi}")
                    for mi, (m0, msz) in enumerate(_M2)]

            def l2_hht(b):
                z1 = ctx[b]["z1"]
                srcs = [(z1[0][:], A1T[b][0][:]),
                        (z1[1][:77, :], A1T[b][1][:77, :]),
                        (U1c, X1T[b][:])]
                ctx[b]["hht"] = mm_chain(b, srcs, (L2C, K1), "hhtc",
                                         relu=True)

            def l2_z2(b):
                hht = ctx[b]["hht"]
                ctx[b]["z2"] = [
                    mm_chain(b, [(hht[:, m0:m0 + msz], W2c)],
                             (msz, L2C), f"z2c_{mi}")
                    for mi, (m0, msz) in enumerate(_M2)]

            def l2_out(b):
                """outc = a1 @ z2c + hh @ U2c, per m-tile: cols 0:21 are the
                s2 head (stays in psum for the softmax), 21:85 the x1e head
                (copied out in f32)."""
                c = ctx[b]
                z2, hht = c["z2"], c["hht"]
                outs, x1e = [], []
                for mi, (m0, msz) in enumerate(_M2):
                    msl = slice(m0, m0 + msz)
                    p = ps.tile([128, L2C], F32, tag="mm", name=f"poc{b}{mi}")
                    nc.tensor.matmul(p[:msz, :], lhsT=A1T[b][0][:, msl],
                                     rhs=z2[0][:], start=True, stop=False)
                    nc.tensor.matmul(p[:msz, :], lhsT=A1T[b][1][:77, msl],
                                     rhs=z2[1][:77, :], start=False, stop=False)
                    nc.tensor.matmul(p[:msz, :], lhsT=hht[:, msl],
                                     rhs=U2c, start=False, stop=True)
                    outs.append(p)
                    xe = l2pool.tile([128, HID], F32, tag=f"x1e_{mi}",
                                     name=f"x1e{b}_{mi}")
                    nc.vector.tensor_copy(out=xe[:msz, :],
                                          in_=p[:msz, K2:L2C])
                    x1e.append(xe)
                c["o"] = outs
                c["xo"] = x1e

            def l2_softmax(b):
                """softmax over K2 on the s2 cols of the outc psums."""
                c = ctx[b]
                sm, smb = [], []
                for mi, (m0, msz) in enumerate(_M2):
                    p = c["o"][mi]
                    nmax = smx.tile([128, 1], F32, tag="nmax", bufs=8,
                                    name=f"l2nm{b}{mi}")
                    nc.vector.reduce_max(out=nmax[:msz], in_=p[:msz, :K2],
                                         axis=mybir.AxisListType.X, negate=True)
                    e = l2pool.tile([128, K2], F32, tag=f"sm2_{mi}",
                                    name=f"sm2{b}_{mi}")
                    ssum = smx.tile([128, 1], F32, tag="l2ssum", bufs=8,
                                    name=f"l2ss{b}{mi}")
                    nc.scalar.activation(out=e[:msz, :], in_=p[:msz, :K2],
                                         func=mybir.ActivationFunctionType.Exp,
                                         bias=nmax[:msz], scale=1.0,
                                         accum_out=ssum[:msz])
                    rinv = smx.tile([128, 1], F32, tag="l2rinv", bufs=8,
                                    name=f"l2ri{b}{mi}")
                    nc.vector.reciprocal(out=rinv[:msz], in_=ssum[:msz])
                    nc.vector.tensor_scalar_mul(out=e[:msz, :], in0=e[:msz, :],
                                                scalar1=rinv[:msz])
                    eb = l2pool.tile([128, K2], BF16, tag=f"sm2b_{mi}",
                                     name=f"sm2b{b}_{mi}")
                    nc.gpsimd.tensor_copy(out=eb[:msz, :], in_=e[:msz, :])
                    sm.append(e)
                    smb.append(eb)
                c["sm2"] = sm
                c["sm2b"] = smb

            def l2_pool_stage(b):
                c = ctx[b]
                sm2, sm2b = c["sm2"], c["sm2b"]
                x1e = c["xo"]
                c["x2t"] = mm_chain(
                    b, [(x1e[0][:], sm2[0][:]), (x1e[1][:77, :], sm2[1][:77, :])],
                    (HID, K2), "x2t", dtype=F32)
                y = []
                for mi, (m0, msz) in enumerate(_M2):
                    msl = slice(m0, m0 + msz)
                    y.append(mm_chain(
                        b, [(A1T[b][0][:, msl], sm2b[0][:]),
                            (A1T[b][1][:77, msl], sm2b[1][:77, :])],
                        (msz, K2), f"y_{mi}", dtype=F32))
                c["a2t"] = mm_chain(
                    b, [(y[0][:], sm2[0][:]), (y[1][:77, :], sm2[1][:77, :])],
                    (K2, K2), "a2t", dtype=F32)

            def l2_final(b):
                c = ctx[b]
                x2t, a2t = c["x2t"], c["a2t"]
                z = mm_chain(b, [(x2t[:], wsb["Wc1f"][:])], (K2, HID), "fz",
                             dtype=F32)
                h2t = mm_chain(b, [(z[:], a2t[:]), (wsb["Uc1f"][:], x2t[:])],
                               (HID, K2), "fh2t", relu=True, dtype=F32)
                z2f = mm_chain(b, [(h2t[:], wsb["Wc2f"][:])], (K2, OUT), "fz2",
                               dtype=F32)
                onodes = mm_chain(b, [(a2t[:], z2f[:]), (h2t[:], wsb["Uc2f"][:])],
                                  (K2, OUT), "fon", dtype=F32)
                pm = ps.tile([OUT, 1], F32, tag="mm", name=f"pm{b}")
                nc.tensor.matmul(pm[:], lhsT=onodes[:], rhs=ones_col[:],
                                 start=True, stop=True)
                nc.scalar.activation(out=out_sb[:, b:b + 1], in_=pm[:],
                                     func=mybir.ActivationFunctionType.Copy,
                                     scale=1.0 / K2)

            # ---------------- emission schedule ----------------
            proj(0, nc.gpsimd)
            a_ = t_pass(0, 0)
            t_copies(0, a_, 0, nc.vector)
            proj(1, nc.gpsimd)
            for u_lo in (4, 8, 12):
                a_ = t_pass(0, u_lo)
                t_copies(0, a_, u_lo, nc.vector)
            epilogue(0)
            # t1 passes with batch-0's level-2 chain woven between
            a_ = t_pass(1, 0)
            t_copies(1, a_, 0, nc.scalar)
            l2_z1(0)
            a_ = t_pass(1, 4)
            t_copies(1, a_, 4, nc.scalar)
            l2_hht(0)
            a_ = t_pass(1, 8)
            t_copies(1, a_, 8, nc.scalar)
            l2_z2(0)
            l2_out(0)
            l2_softmax(0)
            a_ = t_pass(1, 12)
            t_copies(1, a_, 12, nc.scalar)
            epilogue(1)
            # batch-1 level-2 interleaved with batch-0's tail
            l2_pool_stage(0)
            l2_z1(1)
            l2_hht(1)
            l2_final(0)
            l2_z2(1)
            l2_out(1)
            l2_softmax(1)
            l2_pool_stage(1)
            l2_final(1)

            nc.sync.dma_start(out=out[:], in_=out_sb[:])

    _legalize_multiwait(nc)
    return nc


# ---------------------------------------------------------------------------
# Host side
# ---------------------------------------------------------------------------
def _prep_inputs(inputs):
    inp = {k: np.asarray(v) for k, v in inputs.items()}
    sl1 = inp["slice_g1"].astype(np.int64)
    sl2 = inp["slice_g2"].astype(np.int64)
    b1 = inp["batch_g1"].astype(np.int64)
    b2 = inp["batch_g2"].astype(np.int64)
    n1 = np.diff(sl1)
    pos1 = np.arange(inp["x_g1"].shape[0], dtype=np.int64) - sl1[b1]
    pos2 = (np.arange(inp["x_g2"].shape[0], dtype=np.int64) - sl2[b2]
            + n1[b2])

    # packed dense transposed features per batch (g1: cols 0..1151,
    # g2: original cols 1024..2047 stored at offset -1024), bf16
    xt1 = np.zeros((B, IN_DIM, 9 * 128), np.float32)
    xt2 = np.zeros((B, IN_DIM, 8 * 128), np.float32)
    xg1t = inp["x_g1"].T
    xg2t = inp["x_g2"].T
    for b in range(B):
        r1 = slice(sl1[b], sl1[b + 1])
        xt1[b][:, pos1[r1]] = xg1t[:, r1]
        r2 = slice(sl2[b], sl2[b + 1])
        xt2[b][:, pos2[r2] - 1024] = xg2t[:, r2]
    xt1 = xt1.astype(np.float32)
    xt2 = xt2.astype(np.float32)

    # transposed dense adjacency, fp8e4 (1.0 = 0x38), one per batch.
    # layout: [dst, src]; split into trimmed dst-chunks 0..8 / full 8..15
    e1, e2, eh = inp["edge_g1"], inp["edge_g2"], inp["edge_h"]
    eb = np.concatenate([b1[e1[0]], b2[e2[0]], b1[eh[0]]]).astype(np.int64)
    erow = np.concatenate([pos1[e1[0]], pos2[e2[0]], pos1[eh[0]]])
    ecol = np.concatenate([pos1[e1[1]], pos2[e2[1]], pos2[eh[1]]])
    adj_u8 = np.zeros((B, MN, MN), np.uint8)           # [b, dst, src]
    adj_u8[eb, ecol, erow] = 0x38
    adj3 = adj_u8.reshape(B, 16, 128, MN)
    adj1 = np.ascontiguousarray(adj3[:, :8, :, :W1COLS]).view(ml_dtypes.float8_e4m3)
    adj2 = np.ascontiguousarray(adj3[:, 8:, :, :]).view(ml_dtypes.float8_e4m3)

    # projection weights: [205 scores | 3 zero pad | 64 emb] x2, bf16, packed
    wproj = np.zeros((IN_DIM, 2 * PROJ_N), np.float32)
    wproj[:, :K1] = inp["W_pool_g1"]
    wproj[:, HOFF:PROJ_N] = inp["W_emb_g1"]
    wproj[:, PROJ_N:PROJ_N + K1] = inp["W_pool_g2"]
    wproj[:, PROJ_N + HOFF:] = inp["W_emb_g2"]

    # combined level-2 weights: [W1c | U1c | W2c | U2c]; W2c/U2c are
    # 85x85 block-diagonal (s2 head 21 wide, x1e head 64 wide)
    L2Cn = K2 + HID
    wl2cat = np.zeros((L2Cn, 4 * L2Cn), np.float32)
    wl2cat[:HID, 0:K2] = inp["Wp1"]
    wl2cat[:HID, K2:L2Cn] = inp["We1"]
    wl2cat[:HID, L2Cn:L2Cn + K2] = inp["Up1"]
    wl2cat[:HID, L2Cn + K2:2 * L2Cn] = inp["Ue1"]
    wl2cat[:K2, 2 * L2Cn:2 * L2Cn + K2] = inp["Wp2"]
    wl2cat[K2:L2Cn, 2 * L2Cn + K2:3 * L2Cn] = inp["We2"]
    wl2cat[:K2, 3 * L2Cn:3 * L2Cn + K2] = inp["Up2"]
    wl2cat[K2:L2Cn, 3 * L2Cn + K2:4 * L2Cn] = inp["Ue2"]

    wl2f32 = np.concatenate(
        [inp["Wc1"], inp["Uc1"], inp["Wc2"], inp["Uc2"]], axis=1
    ).astype(np.float32)
    shared = dict(
        wl2cat=wl2cat.astype(ml_dtypes.bfloat16),
        wl2f32=wl2f32,
    )
    in_maps = []
    for c in range(NCORES):
        b0, b1_ = c * BPC, c * BPC + 1
        xp0 = np.concatenate([xt1[b0], xt2[b0], wproj], axis=1)
        xp1 = np.concatenate([xt1[b1_], xt2[b1_]], axis=1)
        in_maps.append(dict(
            xp0=np.ascontiguousarray(xp0.astype(ml_dtypes.bfloat16)),
            xp1=np.ascontiguousarray(xp1.astype(ml_dtypes.bfloat16)),
            adj1=np.ascontiguousarray(adj1[slice(b0, b0 + BPC)]),
            adj2=np.ascontiguousarray(adj2[slice(b0, b0 + BPC)]),
            **shared,
        ))
    return in_maps


_NC_CACHE = {}


def run(inputs, trace=False, tmpdir=None):
    if "nc" not in _NC_CACHE:
        _NC_CACHE["nc"] = build_nc()
    nc = _NC_CACHE["nc"]
    in_maps = _prep_inputs(inputs)
    res = run_bass_kernel_spmd(nc, in_maps, list(range(NCORES)),
                               trace=trace, tmpdir=tmpdir)
    y = np.zeros((B, OUT), np.float32)
    for c in range(NCORES):
        o = res.results[c]["out"]       # [OUT, BPC]
        for b in range(BPC):
            y[c * BPC + b] = o[:, b]
    return y, res


def kernel(**inputs):
    y, _ = run(inputs)
    return y
